# revision 2
# baseline (speedup 1.0000x reference)
"""Trainium2 Bass kernel for nn_DistSAGE (3-layer GraphSAGE, mean aggregation).

Strategy: globally-deduplicated layer-wise sharding over the 8 NeuronCores
with two small AllGathers. The host computes the global unique dst sets each
layer actually needs (layer-1 dsts ~1.8K of 2560, layer-0 dsts ~13K of 25600)
— the naive per-subtree recursion computes each layer-0 dst ~2.3x across
cores, so deduplicating globally cuts the dominant layer-0 feature-gather
traffic from ~167MB to ~75MB per core. Each core:
  layer 0: owns a 1/8 contiguous slice of the global padded layer-0 dst list;
           indirect-DMA gathers self + 10 neighbor x rows (f32, 4KB) per dst,
           tree-adds neighbors on DVE, transposes on the PE, matmuls against
           [Wself ; Wneigh/10] into PSUM (+bias via a K=1 ones matmul), relu
           -> its h0 shard in DRAM.
  AllGather h0 shards -> full h0 table (~13MB) on every core.
  layer 1: same over its 1/8 slice of the global layer-1 dst list (D=256),
           gathering from the allgathered h0; AllGather h1 (~2MB).
  layer 2: computes its 64 output logits from the allgathered h1.
The output is dst-major so each core writes its 64x19 logits; the host
concatenates. Indices are positions in the padded global lists, so the
AllGather concatenation order makes position == row index.
"""

import numpy as np

_N0, _N1, _N2, _N3 = 256000, 25600, 2560, 512
_DIN, _DH, _DOUT = 1024, 256, 19
_F0, _F1, _F2 = 10, 10, 5
_NCORES = 8
_P = 128
_OUT_PER_CORE = _N3 // _NCORES  # 64

_compiled = {}


def _build(u0pc, u1pc):
    import concourse.bass as bass
    import concourse.mybir as mybir
    import concourse.tile as tile
    from concourse import bacc
    from concourse.masks import make_identity

    P = _P
    nc = bacc.Bacc(
        "TRN2", target_bir_lowering=False, debug=False, num_devices=_NCORES,
        num_swdge_queues=4,
    )
    f32 = mybir.dt.float32
    i32 = mybir.dt.int32

    x = nc.dram_tensor("x", [_N0, _DIN], f32, kind="ExternalInput")
    gidx0 = nc.dram_tensor("gidx0", [u0pc, _F0 + 1], i32, kind="ExternalInput")
    gidx1 = nc.dram_tensor("gidx1", [u1pc, _F1 + 1], i32, kind="ExternalInput")
    gidx2 = nc.dram_tensor("gidx2", [P, _F2 + 1], i32, kind="ExternalInput")
    wcat0 = nc.dram_tensor("wcat0", [2 * _DIN, _DH], f32, kind="ExternalInput")
    wcat1 = nc.dram_tensor("wcat1", [2 * _DH, _DH], f32, kind="ExternalInput")
    wcat2 = nc.dram_tensor("wcat2", [2 * _DH, _DOUT], f32, kind="ExternalInput")
    bias0 = nc.dram_tensor("bias0", [1, _DH], f32, kind="ExternalInput")
    bias1 = nc.dram_tensor("bias1", [1, _DH], f32, kind="ExternalInput")
    bias2 = nc.dram_tensor("bias2", [1, _DOUT], f32, kind="ExternalInput")
    out = nc.dram_tensor("out", [P, _DOUT], f32, kind="ExternalOutput")

    h0s = nc.dram_tensor("h0shard", [u0pc, _DH], f32, kind="Internal")
    h0f = nc.dram_tensor("h0full", [_NCORES * u0pc, _DH], f32, kind="Internal")
    h1s = nc.dram_tensor("h1shard", [u1pc, _DH], f32, kind="Internal")
    h1f = nc.dram_tensor("h1full", [_NCORES * u1pc, _DH], f32, kind="Internal")

    with tile.TileContext(nc) as tc:
        with (
            tc.tile_pool(name="const", bufs=1) as cpool,
            tc.tile_pool(name="gather", bufs=2) as gpool,
            tc.tile_pool(name="zt", bufs=2) as zpool,
            tc.tile_pool(name="outp", bufs=2) as opool,
            tc.tile_pool(name="psacc", bufs=2, space="PSUM") as psacc,
            tc.tile_pool(name="pstp", bufs=4, space="PSUM") as pstp,
        ):
            ident = cpool.tile([P, P], f32)
            make_identity(nc, ident[:])
            ones = cpool.tile([1, P], f32)
            nc.gpsimd.memset(ones[:], 1.0)

            # resident weights: k-chunk c of wcat lives at wt[:, c*DO:(c+1)*DO]
            def load_w(wdram, kd, do, name):
                wt = cpool.tile([P, kd // P * do], f32, name=name)
                for k in range(kd // P):
                    nc.sync.dma_start(
                        out=wt[:, k * do : (k + 1) * do],
                        in_=wdram[k * P : (k + 1) * P, :],
                    )
                return wt

            wt0 = load_w(wcat0, 2 * _DIN, _DH, "wt0")
            wt1 = load_w(wcat1, 2 * _DH, _DH, "wt1")
            wt2 = load_w(wcat2, 2 * _DH, _DOUT, "wt2")
            bt0 = cpool.tile([1, _DH], f32)
            nc.sync.dma_start(out=bt0[:], in_=bias0[:])
            bt1 = cpool.tile([1, _DH], f32)
            nc.sync.dma_start(out=bt1[:], in_=bias1[:])
            bt2 = cpool.tile([1, _DOUT], f32)
            nc.sync.dma_start(out=bt2[:], in_=bias2[:])

            def layer(src, idx_dram, nd, d, fan, wt, bt, do, relu, dst):
                g_width = (fan + 1) * d
                kc = 2 * d // P
                for c in range(nd // P):
                    idx_t = gpool.tile([P, fan + 1], i32, tag="idx")
                    nc.sync.dma_start(
                        out=idx_t[:], in_=idx_dram[c * P : (c + 1) * P, :]
                    )
                    g = gpool.tile([P, g_width], f32, tag=f"g{d}")
                    for j in range(fan + 1):
                        ins = nc.gpsimd.indirect_dma_start(
                            out=g[:, j * d : (j + 1) * d],
                            out_offset=None,
                            in_=src[:],
                            in_offset=bass.IndirectOffsetOnAxis(
                                ap=idx_t[:, j : j + 1], axis=0
                            ),
                        )
                        # spread gathers over the 4 SWDGE queues so descriptor
                        # generation/drain pipelines instead of serializing
                        if j % 4:
                            ins.ins.queue = f"qPoolDynamic{j % 4}"
                    # tree-sum the fan neighbor rows into g[:, d:2d]
                    if fan == 10:
                        nc.vector.tensor_add(
                            out=g[:, d : 6 * d], in0=g[:, d : 6 * d],
                            in1=g[:, 6 * d : 11 * d],
                        )
                        nc.vector.tensor_add(
                            out=g[:, d : 3 * d], in0=g[:, d : 3 * d],
                            in1=g[:, 3 * d : 5 * d],
                        )
                        nc.vector.tensor_add(
                            out=g[:, d : 2 * d], in0=g[:, d : 2 * d],
                            in1=g[:, 2 * d : 3 * d],
                        )
                        nc.vector.tensor_add(
                            out=g[:, d : 2 * d], in0=g[:, d : 2 * d],
                            in1=g[:, 5 * d : 6 * d],
                        )
                    elif fan == 5:
                        nc.vector.tensor_add(
                            out=g[:, d : 3 * d], in0=g[:, d : 3 * d],
                            in1=g[:, 3 * d : 5 * d],
                        )
                        nc.vector.tensor_add(
                            out=g[:, d : 2 * d], in0=g[:, d : 2 * d],
                            in1=g[:, 2 * d : 3 * d],
                        )
                        nc.vector.tensor_add(
                            out=g[:, d : 2 * d], in0=g[:, d : 2 * d],
                            in1=g[:, 5 * d : 6 * d],
                        )
                    else:
                        raise NotImplementedError(fan)

                    zt = zpool.tile([P, 2 * d], f32, tag=f"zt{d}")
                    for k in range(kc):
                        tp = pstp.tile([P, P], f32, tag="tp")
                        nc.tensor.transpose(
                            out=tp[:], in_=g[:, k * P : (k + 1) * P],
                            identity=ident[:],
                        )
                        nc.vector.tensor_copy(
                            out=zt[:, k * P : (k + 1) * P], in_=tp[:]
                        )

                    acc = psacc.tile([P, do], f32, tag="acc")
                    for k in range(kc):
                        nc.tensor.matmul(
                            out=acc[:],
                            lhsT=zt[:, k * P : (k + 1) * P],
                            rhs=wt[:, k * do : (k + 1) * do],
                            start=(k == 0),
                            stop=False,
                        )
                    nc.tensor.matmul(
                        out=acc[:], lhsT=ones[:], rhs=bt[:], start=False, stop=True
                    )
                    ot = opool.tile([P, do], f32, tag=f"ot{do}")
                    nc.scalar.activation(
                        out=ot[:],
                        in_=acc[:],
                        func=(
                            mybir.ActivationFunctionType.Relu
                            if relu
                            else mybir.ActivationFunctionType.Copy
                        ),
                    )
                    nc.sync.dma_start(out=dst[c * P : (c + 1) * P, :], in_=ot[:])

            grp = [list(range(_NCORES))]

            layer(x, gidx0, u0pc, _DIN, _F0, wt0, bt0, _DH, True, h0s)
            nc.gpsimd.collective_compute(
                "AllGather", mybir.AluOpType.bypass, replica_groups=grp,
                ins=[h0s[:].opt()], outs=[h0f[:].opt()],
            )
            layer(h0f, gidx1, u1pc, _DH, _F1, wt1, bt1, _DH, True, h1s)
            nc.gpsimd.collective_compute(
                "AllGather", mybir.AluOpType.bypass, replica_groups=grp,
                ins=[h1s[:].opt()], outs=[h1f[:].opt()],
            )
            layer(h1f, gidx2, P, _DH, _F2, wt2, bt2, _DOUT, False, out)

    nc.compile()
    return nc


def _plan(x, nbr0, nbr1, nbr2, weights):
    """Host-side sharding: global dedup + per-core slices of padded lists."""
    C, P = _NCORES, _P
    nbr0 = np.asarray(nbr0, dtype=np.int64)
    nbr1 = np.asarray(nbr1, dtype=np.int64)
    nbr2 = np.asarray(nbr2, dtype=np.int64)

    out_all = np.arange(_N3, dtype=np.int64)
    # global layer-1 dst list: the 512 output nodes first (so position ==
    # node id for them), then the other layer-1 nodes any subtree samples
    need1 = np.concatenate([out_all, np.setdiff1d(np.unique(nbr2), out_all)])
    u1 = len(need1)
    u1pc = -(-u1 // (C * P)) * P
    need1p = np.full(C * u1pc, need1[0], np.int64)
    need1p[:u1] = need1
    inv1 = np.full(_N2, -1, np.int64)
    inv1[need1] = np.arange(u1)

    # global layer-0 dst list: everything layer 1 touches (self + neighbors)
    need0 = np.union1d(need1, np.unique(nbr1[need1]))
    u0 = len(need0)
    u0pc = -(-u0 // (C * P)) * P
    need0p = np.full(C * u0pc, need0[0], np.int64)
    need0p[:u0] = need0
    inv0 = np.full(_N1, -1, np.int64)
    inv0[need0] = np.arange(u0)

    gidx0_all = np.empty((C * u0pc, _F0 + 1), np.int32)
    gidx0_all[:, 0] = need0p
    gidx0_all[:, 1:] = nbr0[need0p]

    gidx1_all = np.empty((C * u1pc, _F1 + 1), np.int32)
    gidx1_all[:, 0] = inv0[need1p]
    gidx1_all[:, 1:] = inv0[nbr1[need1p]]

    ids = out_all.reshape(C, _OUT_PER_CORE)
    gidx2_all = np.zeros((C, P, _F2 + 1), np.int32)
    gidx2_all[:, : _OUT_PER_CORE, 0] = inv1[ids]
    gidx2_all[:, : _OUT_PER_CORE, 1:] = inv1[nbr2[ids]]

    wcat0 = np.concatenate(
        [weights["Wself0"], weights["Wneigh0"] / _F0], axis=0
    ).astype(np.float32)
    wcat1 = np.concatenate(
        [weights["Wself1"], weights["Wneigh1"] / _F1], axis=0
    ).astype(np.float32)
    wcat2 = np.concatenate(
        [weights["Wself2"], weights["Wneigh2"] / _F2], axis=0
    ).astype(np.float32)
    b0 = weights["b0"].reshape(1, -1).astype(np.float32)
    b1 = weights["b1"].reshape(1, -1).astype(np.float32)
    b2 = weights["b2"].reshape(1, -1).astype(np.float32)

    in_maps = []
    for k in range(C):
        in_maps.append(
            {
                "x": x,
                "gidx0": gidx0_all[k * u0pc : (k + 1) * u0pc],
                "gidx1": gidx1_all[k * u1pc : (k + 1) * u1pc],
                "gidx2": gidx2_all[k],
                "wcat0": wcat0,
                "wcat1": wcat1,
                "wcat2": wcat2,
                "bias0": b0,
                "bias1": b1,
                "bias2": b2,
            }
        )
    return in_maps, u0pc, u1pc


def _prepare(**inputs):
    x = np.ascontiguousarray(np.asarray(inputs["x"], dtype=np.float32))
    nbr0 = np.asarray(inputs["nbr0"])
    nbr1 = np.asarray(inputs["nbr1"])
    nbr2 = np.asarray(inputs["nbr2"])
    weights = {
        k: np.asarray(inputs[k], dtype=np.float32)
        for k in (
            "Wself0", "Wneigh0", "b0",
            "Wself1", "Wneigh1", "b1",
            "Wself2", "Wneigh2", "b2",
        )
    }
    in_maps, u0pc, u1pc = _plan(x, nbr0, nbr1, nbr2, weights)
    key = (u0pc, u1pc)
    if key not in _compiled:
        _compiled[key] = _build(u0pc, u1pc)
    return _compiled[key], in_maps


def kernel(**inputs) -> np.ndarray:
    from concourse.bass_utils import run_bass_kernel_spmd

    nc, in_maps = _prepare(**inputs)
    res = run_bass_kernel_spmd(nc, in_maps, core_ids=list(range(_NCORES)))
    out = np.concatenate(
        [res.results[k]["out"][:_OUT_PER_CORE] for k in range(_NCORES)], axis=0
    )
    return out.astype(np.float32)


# revision 4
# speedup vs baseline: 1.6588x; 1.6588x over previous
"""Trainium2 Bass kernel for nn_DistSAGE (3-layer GraphSAGE, mean aggregation).

v4: bf16 datapath + globally-deduplicated layer-0 sharding with ONE AllGather.

The host computes the global unique layer-0 dst set (~13K of 25600; the naive
per-subtree recursion would compute each ~2.3x across cores) and shards it
evenly. Features, weights and hidden tables are bf16 (PSUM accumulation stays
f32; final logits f32), halving the dominant gather traffic. Layer 0 writes
each core's h0 shard; one AllGather rebuilds the full h0 table (~7MB bf16) on
every core; layers 1-2 then run per-subtree locally (each core only needs
~350 layer-1 rows for its own 64 outputs) with no further collectives.

Per-chunk machinery (all layers): indirect-DMA gather self + fan neighbor
rows per dst, tree-add neighbors on DVE, transpose [dst, feat] -> [feat, dst]
on the PE, matmul against [Wself ; Wneigh/fan] into PSUM (+bias via a K=1
ones matmul), relu via the scalar engine. Output is dst-major; each core
writes its 64x19 logits and the host concatenates.
"""

import numpy as np
import ml_dtypes

_BF16 = ml_dtypes.bfloat16

_N0, _N1, _N2, _N3 = 256000, 25600, 2560, 512
_DIN, _DH, _DOUT = 1024, 256, 19
_F0, _F1, _F2 = 10, 10, 5
_NCORES = 8
_P = 128
_OUT_PER_CORE = _N3 // _NCORES  # 64

_compiled = {}


def _build(u0pc, u1pc):
    import concourse.bass as bass
    import concourse.mybir as mybir
    import concourse.tile as tile
    from concourse import bacc
    from concourse.masks import make_identity

    P = _P
    nc = bacc.Bacc(
        "TRN2", target_bir_lowering=False, debug=False, num_devices=_NCORES,
        num_swdge_queues=4,
    )
    f32 = mybir.dt.float32
    bf16 = mybir.dt.bfloat16
    i32 = mybir.dt.int32

    x = nc.dram_tensor("x", [_N0, _DIN], bf16, kind="ExternalInput")
    gidx0 = nc.dram_tensor("gidx0", [u0pc, _F0 + 1], i32, kind="ExternalInput")
    gidx1 = nc.dram_tensor("gidx1", [u1pc, _F1 + 1], i32, kind="ExternalInput")
    gidx2 = nc.dram_tensor("gidx2", [P, _F2 + 1], i32, kind="ExternalInput")
    wcat0 = nc.dram_tensor("wcat0", [2 * _DIN, _DH], bf16, kind="ExternalInput")
    wcat1 = nc.dram_tensor("wcat1", [2 * _DH, _DH], bf16, kind="ExternalInput")
    wcat2 = nc.dram_tensor("wcat2", [2 * _DH, _DOUT], bf16, kind="ExternalInput")
    bias0 = nc.dram_tensor("bias0", [1, _DH], bf16, kind="ExternalInput")
    bias1 = nc.dram_tensor("bias1", [1, _DH], bf16, kind="ExternalInput")
    bias2 = nc.dram_tensor("bias2", [1, _DOUT], bf16, kind="ExternalInput")
    out = nc.dram_tensor("out", [P, _DOUT], f32, kind="ExternalOutput")

    h0s = nc.dram_tensor("h0shard", [u0pc, _DH], bf16, kind="Internal")
    h0f = nc.dram_tensor(
        "h0full", [_NCORES * u0pc, _DH], bf16, kind="Internal",
        addr_space="Shared",
    )
    h1 = nc.dram_tensor("h1local", [u1pc, _DH], bf16, kind="Internal")

    with tile.TileContext(nc) as tc:
        with (
            tc.tile_pool(name="const", bufs=1) as cpool,
            tc.tile_pool(name="gather", bufs=2) as gpool,
            tc.tile_pool(name="zt", bufs=2) as zpool,
            tc.tile_pool(name="outp", bufs=2) as opool,
            tc.tile_pool(name="psacc", bufs=2, space="PSUM") as psacc,
            tc.tile_pool(name="pstp", bufs=4, space="PSUM") as pstp,
        ):
            ident = cpool.tile([P, P], bf16)
            make_identity(nc, ident[:])
            ones = cpool.tile([1, P], bf16)
            nc.gpsimd.memset(ones[:], 1.0)

            # resident weights: k-chunk c of wcat lives at wt[:, c*DO:(c+1)*DO]
            def load_w(wdram, kd, do, name):
                wt = cpool.tile([P, kd // P * do], bf16, name=name)
                for k in range(kd // P):
                    nc.sync.dma_start(
                        out=wt[:, k * do : (k + 1) * do],
                        in_=wdram[k * P : (k + 1) * P, :],
                    )
                return wt

            wt0 = load_w(wcat0, 2 * _DIN, _DH, "wt0")
            wt1 = load_w(wcat1, 2 * _DH, _DH, "wt1")
            wt2 = load_w(wcat2, 2 * _DH, _DOUT, "wt2")
            bt0 = cpool.tile([1, _DH], bf16)
            nc.sync.dma_start(out=bt0[:], in_=bias0[:])
            bt1 = cpool.tile([1, _DH], bf16)
            nc.sync.dma_start(out=bt1[:], in_=bias1[:])
            bt2 = cpool.tile([1, _DOUT], bf16)
            nc.sync.dma_start(out=bt2[:], in_=bias2[:])

            def layer(src, idx_dram, nd, d, fan, wt, bt, do, relu, dst, odt):
                g_width = (fan + 1) * d
                kc = 2 * d // P
                for c in range(nd // P):
                    idx_t = gpool.tile([P, fan + 1], i32, tag="idx")
                    nc.sync.dma_start(
                        out=idx_t[:], in_=idx_dram[c * P : (c + 1) * P, :]
                    )
                    g = gpool.tile([P, g_width], bf16, tag=f"g{d}")
                    for j in range(fan + 1):
                        ins = nc.gpsimd.indirect_dma_start(
                            out=g[:, j * d : (j + 1) * d],
                            out_offset=None,
                            in_=src[:],
                            in_offset=bass.IndirectOffsetOnAxis(
                                ap=idx_t[:, j : j + 1], axis=0
                            ),
                        )
                        # spread gathers over the 4 SWDGE queues so descriptor
                        # generation/drain pipelines instead of serializing
                        if j % 4:
                            ins.ins.queue = f"qPoolDynamic{j % 4}"
                    # tree-sum the fan neighbor rows into g[:, d:2d]
                    if fan == 10:
                        nc.vector.tensor_add(
                            out=g[:, d : 6 * d], in0=g[:, d : 6 * d],
                            in1=g[:, 6 * d : 11 * d],
                        )
                        nc.vector.tensor_add(
                            out=g[:, d : 3 * d], in0=g[:, d : 3 * d],
                            in1=g[:, 3 * d : 5 * d],
                        )
                        nc.vector.tensor_add(
                            out=g[:, d : 2 * d], in0=g[:, d : 2 * d],
                            in1=g[:, 2 * d : 3 * d],
                        )
                        nc.vector.tensor_add(
                            out=g[:, d : 2 * d], in0=g[:, d : 2 * d],
                            in1=g[:, 5 * d : 6 * d],
                        )
                    elif fan == 5:
                        nc.vector.tensor_add(
                            out=g[:, d : 3 * d], in0=g[:, d : 3 * d],
                            in1=g[:, 3 * d : 5 * d],
                        )
                        nc.vector.tensor_add(
                            out=g[:, d : 2 * d], in0=g[:, d : 2 * d],
                            in1=g[:, 2 * d : 3 * d],
                        )
                        nc.vector.tensor_add(
                            out=g[:, d : 2 * d], in0=g[:, d : 2 * d],
                            in1=g[:, 5 * d : 6 * d],
                        )
                    else:
                        raise NotImplementedError(fan)

                    zt = zpool.tile([P, 2 * d], bf16, tag=f"zt{d}")
                    for k in range(kc):
                        tp = pstp.tile([P, P], bf16, tag="tp")
                        nc.tensor.transpose(
                            out=tp[:], in_=g[:, k * P : (k + 1) * P],
                            identity=ident[:],
                        )
                        nc.vector.tensor_copy(
                            out=zt[:, k * P : (k + 1) * P], in_=tp[:]
                        )

                    acc = psacc.tile([P, do], f32, tag="acc")
                    for k in range(kc):
                        nc.tensor.matmul(
                            out=acc[:],
                            lhsT=zt[:, k * P : (k + 1) * P],
                            rhs=wt[:, k * do : (k + 1) * do],
                            start=(k == 0),
                            stop=False,
                        )
                    nc.tensor.matmul(
                        out=acc[:], lhsT=ones[:], rhs=bt[:], start=False, stop=True
                    )
                    ot = opool.tile([P, do], odt, tag=f"ot{do}{odt}")
                    nc.scalar.activation(
                        out=ot[:],
                        in_=acc[:],
                        func=(
                            mybir.ActivationFunctionType.Relu
                            if relu
                            else mybir.ActivationFunctionType.Copy
                        ),
                    )
                    nc.sync.dma_start(out=dst[c * P : (c + 1) * P, :], in_=ot[:])

            layer(x, gidx0, u0pc, _DIN, _F0, wt0, bt0, _DH, True, h0s, bf16)
            nc.gpsimd.collective_compute(
                "AllGather", mybir.AluOpType.bypass,
                replica_groups=[list(range(_NCORES))],
                ins=[h0s[:].opt()], outs=[h0f[:].opt()],
            )
            layer(h0f, gidx1, u1pc, _DH, _F1, wt1, bt1, _DH, True, h1, bf16)
            layer(h1, gidx2, P, _DH, _F2, wt2, bt2, _DOUT, False, out, f32)

    nc.compile()
    return nc


def _plan(x, nbr0, nbr1, nbr2, weights):
    """Host-side sharding: global layer-0 dedup, per-subtree layers 1-2."""
    C, P = _NCORES, _P
    nbr0 = np.asarray(nbr0, dtype=np.int64)
    nbr1 = np.asarray(nbr1, dtype=np.int64)
    nbr2 = np.asarray(nbr2, dtype=np.int64)

    # per-core layer-1 dst sets (own 64 outputs' self + sampled neighbors)
    need1_k = []
    for k in range(C):
        ids = np.arange(k * _OUT_PER_CORE, (k + 1) * _OUT_PER_CORE)
        need1_k.append(np.union1d(ids, nbr2[ids].ravel()))
    u1pc = -(-max(len(n) for n in need1_k) // P) * P

    # global layer-0 dst set: everything any core's layer 1 touches
    need1_glob = np.unique(np.concatenate(need1_k))
    need0 = np.union1d(need1_glob, np.unique(nbr1[need1_glob]))
    u0 = len(need0)
    u0pc = -(-u0 // (C * P)) * P
    need0p = np.full(C * u0pc, need0[0], np.int64)
    need0p[:u0] = need0
    inv0 = np.full(_N1, -1, np.int64)
    inv0[need0] = np.arange(u0)

    gidx0_all = np.empty((C * u0pc, _F0 + 1), np.int32)
    gidx0_all[:, 0] = need0p
    gidx0_all[:, 1:] = nbr0[need0p]

    bf = _BF16
    wcat0 = np.concatenate(
        [weights["Wself0"], weights["Wneigh0"] / _F0], axis=0
    ).astype(bf)
    wcat1 = np.concatenate(
        [weights["Wself1"], weights["Wneigh1"] / _F1], axis=0
    ).astype(bf)
    wcat2 = np.concatenate(
        [weights["Wself2"], weights["Wneigh2"] / _F2], axis=0
    ).astype(bf)
    b0 = weights["b0"].reshape(1, -1).astype(bf)
    b1 = weights["b1"].reshape(1, -1).astype(bf)
    b2 = weights["b2"].reshape(1, -1).astype(bf)
    xb = np.ascontiguousarray(x.astype(bf))

    in_maps = []
    for k in range(C):
        n1 = need1_k[k]
        n1p = np.full(u1pc, n1[0], np.int64)
        n1p[: len(n1)] = n1
        inv1 = np.full(_N2, -1, np.int64)
        inv1[n1] = np.arange(len(n1))

        gidx1 = np.empty((u1pc, _F1 + 1), np.int32)
        gidx1[:, 0] = inv0[n1p]
        gidx1[:, 1:] = inv0[nbr1[n1p]]

        ids = np.arange(k * _OUT_PER_CORE, (k + 1) * _OUT_PER_CORE)
        gidx2 = np.zeros((P, _F2 + 1), np.int32)
        gidx2[: _OUT_PER_CORE, 0] = inv1[ids]
        gidx2[: _OUT_PER_CORE, 1:] = inv1[nbr2[ids]]

        in_maps.append(
            {
                "x": xb,
                "gidx0": gidx0_all[k * u0pc : (k + 1) * u0pc],
                "gidx1": gidx1,
                "gidx2": gidx2,
                "wcat0": wcat0,
                "wcat1": wcat1,
                "wcat2": wcat2,
                "bias0": b0,
                "bias1": b1,
                "bias2": b2,
            }
        )
    return in_maps, u0pc, u1pc


def _prepare(**inputs):
    x = np.ascontiguousarray(np.asarray(inputs["x"], dtype=np.float32))
    nbr0 = np.asarray(inputs["nbr0"])
    nbr1 = np.asarray(inputs["nbr1"])
    nbr2 = np.asarray(inputs["nbr2"])
    weights = {
        k: np.asarray(inputs[k], dtype=np.float32)
        for k in (
            "Wself0", "Wneigh0", "b0",
            "Wself1", "Wneigh1", "b1",
            "Wself2", "Wneigh2", "b2",
        )
    }
    in_maps, u0pc, u1pc = _plan(x, nbr0, nbr1, nbr2, weights)
    key = (u0pc, u1pc)
    if key not in _compiled:
        _compiled[key] = _build(u0pc, u1pc)
    return _compiled[key], in_maps


def kernel(**inputs) -> np.ndarray:
    from concourse.bass_utils import run_bass_kernel_spmd

    nc, in_maps = _prepare(**inputs)
    res = run_bass_kernel_spmd(nc, in_maps, core_ids=list(range(_NCORES)))
    out = np.concatenate(
        [res.results[k]["out"][:_OUT_PER_CORE] for k in range(_NCORES)], axis=0
    )
    return out.astype(np.float32)


# revision 7
# speedup vs baseline: 2.1580x; 1.3010x over previous
"""Trainium2 Bass kernel for nn_DistSAGE (3-layer GraphSAGE, mean aggregation).

v4: bf16 datapath + globally-deduplicated layer-0 sharding with ONE AllGather.

The host computes the global unique layer-0 dst set (~13K of 25600; the naive
per-subtree recursion would compute each ~2.3x across cores) and shards it
evenly. Features, weights and hidden tables are bf16 (PSUM accumulation stays
f32; final logits f32), halving the dominant gather traffic. Layer 0 writes
each core's h0 shard; one AllGather rebuilds the full h0 table (~7MB bf16) on
every core; layers 1-2 then run per-subtree locally (each core only needs
~350 layer-1 rows for its own 64 outputs) with no further collectives.

Per-chunk machinery (all layers): indirect-DMA gather self + fan neighbor
rows per dst, tree-add neighbors on DVE, transpose [dst, feat] -> [feat, dst]
on the PE, matmul against [Wself ; Wneigh/fan] into PSUM (+bias via a K=1
ones matmul), relu via the scalar engine. Output is dst-major; each core
writes its 64x19 logits and the host concatenates.
"""

import numpy as np
import ml_dtypes

_BF16 = ml_dtypes.bfloat16

_N0, _N1, _N2, _N3 = 256000, 25600, 2560, 512
_DIN, _DH, _DOUT = 1024, 256, 19
_F0, _F1, _F2 = 10, 10, 5
_NCORES = 8
_P = 128
_OUT_PER_CORE = _N3 // _NCORES  # 64

_compiled = {}


def _build(u0pc, u1pc):
    import concourse.bass as bass
    import concourse.mybir as mybir
    import concourse.tile as tile
    from concourse import bacc
    from concourse.masks import make_identity

    P = _P
    nc = bacc.Bacc(
        "TRN2", target_bir_lowering=False, debug=False, num_devices=_NCORES,
        num_swdge_queues=4,
    )
    f32 = mybir.dt.float32
    bf16 = mybir.dt.bfloat16
    i32 = mybir.dt.int32

    x = nc.dram_tensor("x", [_N0, _DIN], bf16, kind="ExternalInput")
    gidx0 = nc.dram_tensor("gidx0", [u0pc, _F0 + 1], i32, kind="ExternalInput")
    gidx1 = nc.dram_tensor("gidx1", [u1pc, _F1 + 1], i32, kind="ExternalInput")
    gidx2 = nc.dram_tensor("gidx2", [P, _F2 + 1], i32, kind="ExternalInput")
    wcat0 = nc.dram_tensor("wcat0", [2 * _DIN, _DH], bf16, kind="ExternalInput")
    wcat1 = nc.dram_tensor("wcat1", [2 * _DH, _DH], bf16, kind="ExternalInput")
    wcat2 = nc.dram_tensor("wcat2", [2 * _DH, _DOUT], bf16, kind="ExternalInput")
    bias0 = nc.dram_tensor("bias0", [1, _DH], bf16, kind="ExternalInput")
    bias1 = nc.dram_tensor("bias1", [1, _DH], bf16, kind="ExternalInput")
    bias2 = nc.dram_tensor("bias2", [1, _DOUT], bf16, kind="ExternalInput")
    out = nc.dram_tensor("out", [P, _DOUT], f32, kind="ExternalOutput")

    h0s = nc.dram_tensor("h0shard", [u0pc, _DH], bf16, kind="Internal")
    h0f = nc.dram_tensor(
        "h0full", [_NCORES * u0pc, _DH], bf16, kind="Internal",
        addr_space="Shared",
    )
    h1 = nc.dram_tensor("h1local", [u1pc, _DH], bf16, kind="Internal")

    with tile.TileContext(nc) as tc:
        with (
            tc.tile_pool(name="const", bufs=1) as cpool,
            tc.tile_pool(name="gather", bufs=3) as gpool,
            tc.tile_pool(name="zt", bufs=2) as zpool,
            tc.tile_pool(name="outp", bufs=2) as opool,
            tc.tile_pool(name="psacc", bufs=2, space="PSUM") as psacc,
            tc.tile_pool(name="pstp", bufs=4, space="PSUM") as pstp,
        ):
            ident = cpool.tile([P, P], bf16)
            make_identity(nc, ident[:])
            ones = cpool.tile([1, P], bf16)
            nc.gpsimd.memset(ones[:], 1.0)

            # resident weights: k-chunk c of wcat lives at wt[:, c*DO:(c+1)*DO]
            def load_w(wdram, kd, do, name):
                wt = cpool.tile([P, kd // P * do], bf16, name=name)
                for k in range(kd // P):
                    nc.sync.dma_start(
                        out=wt[:, k * do : (k + 1) * do],
                        in_=wdram[k * P : (k + 1) * P, :],
                    )
                return wt

            wt0 = load_w(wcat0, 2 * _DIN, _DH, "wt0")
            wt1 = load_w(wcat1, 2 * _DH, _DH, "wt1")
            wt2 = load_w(wcat2, 2 * _DH, _DOUT, "wt2")
            bt0 = cpool.tile([1, _DH], bf16)
            nc.sync.dma_start(out=bt0[:], in_=bias0[:])
            bt1 = cpool.tile([1, _DH], bf16)
            nc.sync.dma_start(out=bt1[:], in_=bias1[:])
            bt2 = cpool.tile([1, _DOUT], bf16)
            nc.sync.dma_start(out=bt2[:], in_=bias2[:])

            def layer(src, idx_dram, nd, d, fan, wt, bt, do, relu, dst, odt):
                g_width = (fan + 1) * d
                kc = 2 * d // P
                nch = nd // P
                # preload every chunk's index block once so per-chunk gathers
                # never wait on an idx DMA round trip
                idx_all = cpool.tile([P, nch * (fan + 1)], i32, name=f"ix{d}{nch}")
                for c in range(nch):
                    nc.sync.dma_start(
                        out=idx_all[:, c * (fan + 1) : (c + 1) * (fan + 1)],
                        in_=idx_dram[c * P : (c + 1) * P, :],
                    )
                for c in range(nch):
                    idx_t = idx_all[:, c * (fan + 1) : (c + 1) * (fan + 1)]
                    g = gpool.tile([P, g_width], bf16, tag=f"g{d}")
                    for j in range(fan + 1):
                        ins = nc.gpsimd.indirect_dma_start(
                            out=g[:, j * d : (j + 1) * d],
                            out_offset=None,
                            in_=src[:],
                            in_offset=bass.IndirectOffsetOnAxis(
                                ap=idx_t[:, j : j + 1], axis=0
                            ),
                        )
                        # spread gathers over the 4 SWDGE queues so descriptor
                        # generation/drain pipelines instead of serializing
                        if j % 4:
                            ins.ins.queue = f"qPoolDynamic{j % 4}"
                    # tree-sum the fan neighbor rows into g[:, d:2d]
                    if fan == 10:
                        nc.vector.tensor_add(
                            out=g[:, d : 6 * d], in0=g[:, d : 6 * d],
                            in1=g[:, 6 * d : 11 * d],
                        )
                        nc.vector.tensor_add(
                            out=g[:, d : 3 * d], in0=g[:, d : 3 * d],
                            in1=g[:, 3 * d : 5 * d],
                        )
                        nc.vector.tensor_add(
                            out=g[:, d : 2 * d], in0=g[:, d : 2 * d],
                            in1=g[:, 2 * d : 3 * d],
                        )
                        nc.vector.tensor_add(
                            out=g[:, d : 2 * d], in0=g[:, d : 2 * d],
                            in1=g[:, 5 * d : 6 * d],
                        )
                    elif fan == 5:
                        nc.vector.tensor_add(
                            out=g[:, d : 3 * d], in0=g[:, d : 3 * d],
                            in1=g[:, 3 * d : 5 * d],
                        )
                        nc.vector.tensor_add(
                            out=g[:, d : 2 * d], in0=g[:, d : 2 * d],
                            in1=g[:, 2 * d : 3 * d],
                        )
                        nc.vector.tensor_add(
                            out=g[:, d : 2 * d], in0=g[:, d : 2 * d],
                            in1=g[:, 5 * d : 6 * d],
                        )
                    else:
                        raise NotImplementedError(fan)

                    zt = zpool.tile([P, 2 * d], bf16, tag=f"zt{d}")
                    for k in range(kc):
                        tp = pstp.tile([P, P], bf16, tag="tp")
                        nc.tensor.transpose(
                            out=tp[:], in_=g[:, k * P : (k + 1) * P],
                            identity=ident[:],
                        )
                        # scalar engine drains PSUM transposes; DVE stays on
                        # the neighbor tree-adds
                        nc.scalar.activation(
                            out=zt[:, k * P : (k + 1) * P], in_=tp[:],
                            func=mybir.ActivationFunctionType.Copy,
                        )

                    acc = psacc.tile([P, do], f32, tag="acc")
                    for k in range(kc):
                        nc.tensor.matmul(
                            out=acc[:],
                            lhsT=zt[:, k * P : (k + 1) * P],
                            rhs=wt[:, k * do : (k + 1) * do],
                            start=(k == 0),
                            stop=False,
                        )
                    nc.tensor.matmul(
                        out=acc[:], lhsT=ones[:], rhs=bt[:], start=False, stop=True
                    )
                    ot = opool.tile([P, do], odt, tag=f"ot{do}{odt}")
                    nc.scalar.activation(
                        out=ot[:],
                        in_=acc[:],
                        func=(
                            mybir.ActivationFunctionType.Relu
                            if relu
                            else mybir.ActivationFunctionType.Copy
                        ),
                    )
                    nc.sync.dma_start(out=dst[c * P : (c + 1) * P, :], in_=ot[:])

            layer(x, gidx0, u0pc, _DIN, _F0, wt0, bt0, _DH, True, h0s, bf16)
            nc.gpsimd.collective_compute(
                "AllGather", mybir.AluOpType.bypass,
                replica_groups=[list(range(_NCORES))],
                ins=[h0s[:].opt()], outs=[h0f[:].opt()],
            )
            layer(h0f, gidx1, u1pc, _DH, _F1, wt1, bt1, _DH, True, h1, bf16)
            layer(h1, gidx2, P, _DH, _F2, wt2, bt2, _DOUT, False, out, f32)

    nc.compile()
    return nc


def _plan(x, nbr0, nbr1, nbr2, weights):
    """Host-side sharding: global layer-0 dedup, per-subtree layers 1-2."""
    C, P = _NCORES, _P
    nbr0 = np.asarray(nbr0, dtype=np.int64)
    nbr1 = np.asarray(nbr1, dtype=np.int64)
    nbr2 = np.asarray(nbr2, dtype=np.int64)

    # per-core layer-1 dst sets (own 64 outputs' self + sampled neighbors)
    need1_k = []
    for k in range(C):
        ids = np.arange(k * _OUT_PER_CORE, (k + 1) * _OUT_PER_CORE)
        need1_k.append(np.union1d(ids, nbr2[ids].ravel()))
    u1pc = -(-max(len(n) for n in need1_k) // P) * P

    # global layer-0 dst set: everything any core's layer 1 touches
    need1_glob = np.unique(np.concatenate(need1_k))
    need0 = np.union1d(need1_glob, np.unique(nbr1[need1_glob]))
    u0 = len(need0)
    u0pc = -(-u0 // (C * P)) * P
    need0p = np.full(C * u0pc, need0[0], np.int64)
    need0p[:u0] = need0
    inv0 = np.full(_N1, -1, np.int64)
    inv0[need0] = np.arange(u0)

    gidx0_all = np.empty((C * u0pc, _F0 + 1), np.int32)
    gidx0_all[:, 0] = need0p
    gidx0_all[:, 1:] = nbr0[need0p]

    bf = _BF16
    wcat0 = np.concatenate(
        [weights["Wself0"], weights["Wneigh0"] / _F0], axis=0
    ).astype(bf)
    wcat1 = np.concatenate(
        [weights["Wself1"], weights["Wneigh1"] / _F1], axis=0
    ).astype(bf)
    wcat2 = np.concatenate(
        [weights["Wself2"], weights["Wneigh2"] / _F2], axis=0
    ).astype(bf)
    b0 = weights["b0"].reshape(1, -1).astype(bf)
    b1 = weights["b1"].reshape(1, -1).astype(bf)
    b2 = weights["b2"].reshape(1, -1).astype(bf)
    xb = np.ascontiguousarray(x.astype(bf))

    in_maps = []
    for k in range(C):
        n1 = need1_k[k]
        n1p = np.full(u1pc, n1[0], np.int64)
        n1p[: len(n1)] = n1
        inv1 = np.full(_N2, -1, np.int64)
        inv1[n1] = np.arange(len(n1))

        gidx1 = np.empty((u1pc, _F1 + 1), np.int32)
        gidx1[:, 0] = inv0[n1p]
        gidx1[:, 1:] = inv0[nbr1[n1p]]

        ids = np.arange(k * _OUT_PER_CORE, (k + 1) * _OUT_PER_CORE)
        gidx2 = np.zeros((P, _F2 + 1), np.int32)
        gidx2[: _OUT_PER_CORE, 0] = inv1[ids]
        gidx2[: _OUT_PER_CORE, 1:] = inv1[nbr2[ids]]

        in_maps.append(
            {
                "x": xb,
                "gidx0": gidx0_all[k * u0pc : (k + 1) * u0pc],
                "gidx1": gidx1,
                "gidx2": gidx2,
                "wcat0": wcat0,
                "wcat1": wcat1,
                "wcat2": wcat2,
                "bias0": b0,
                "bias1": b1,
                "bias2": b2,
            }
        )
    return in_maps, u0pc, u1pc


def _prepare(**inputs):
    x = np.ascontiguousarray(np.asarray(inputs["x"], dtype=np.float32))
    nbr0 = np.asarray(inputs["nbr0"])
    nbr1 = np.asarray(inputs["nbr1"])
    nbr2 = np.asarray(inputs["nbr2"])
    weights = {
        k: np.asarray(inputs[k], dtype=np.float32)
        for k in (
            "Wself0", "Wneigh0", "b0",
            "Wself1", "Wneigh1", "b1",
            "Wself2", "Wneigh2", "b2",
        )
    }
    in_maps, u0pc, u1pc = _plan(x, nbr0, nbr1, nbr2, weights)
    key = (u0pc, u1pc)
    if key not in _compiled:
        _compiled[key] = _build(u0pc, u1pc)
    return _compiled[key], in_maps


def kernel(**inputs) -> np.ndarray:
    from concourse.bass_utils import run_bass_kernel_spmd

    nc, in_maps = _prepare(**inputs)
    res = run_bass_kernel_spmd(nc, in_maps, core_ids=list(range(_NCORES)))
    out = np.concatenate(
        [res.results[k]["out"][:_OUT_PER_CORE] for k in range(_NCORES)], axis=0
    )
    return out.astype(np.float32)
